# revision 19
# baseline (speedup 1.0000x reference)
"""Trainium2 Bass kernel for nn_ContinuousCRF (mean-field CRF, 96x96 image, 3 classes).

Key algebraic identity: the dense N^2 pairwise matrix (N=9216) is pure
geometry -- pairwise[n,m] = exp(-dist(n,m)) * (dist<=5), diag=0 -- so
`messages = pairwise @ q` is exactly an 11x11 spatial convolution with 80
nonzero taps.  We implement that conv as 11 accumulating TensorE matmuls
(one banded [96,96] matrix per row offset dy, contracting over the x axis),
plus a 12th identity matmul that adds the unary term into the same PSUM
accumulation.  Softmax over the 3 classes runs on ACT (exp) + DVE
(two adds for the class sum, reciprocal, one broadcast multiply).

Layout on chip: partitions = x (96), free dims = (c=3, y-slice); the q tile
is y-padded by 5 on both sides so all 11 shifted matmul reads are in-bounds.
q, band, unary and e are fp16: 2-byte matmul operands keep the PE at 1
cycle/row even for narrow region outputs, and packed fp16 SBUF operands
unlock the DVE 2x mode (~0.05% quantization, inside the 2e-2 gate).

I/O via SWDGE prepare/trigger: HWDGE descriptor generation costs 625ns on an
exclusive device plus a 650ns DGE start delay per DMA.  Instead, descriptor
generation for both input gathers and the output scatter runs on the Pool
engine in the preamble (994ns each, off the critical path), and a cheap
trigger_dma fires each transfer the moment its dependency lands.  Inputs are
two row-gathers (96 rows each): [unary(3), ident, band j0..j3] first (gates
the stage-0 softmax), [band j4..j10, pad] second (only gates the 6th conv
tap, ~200ns later).  The output is one dma_scatter_add into a pre-zeroed
DRAM buffer (zeroed by a plain HWDGE DMA in the otherwise-idle preamble),
fired by a trigger that waits only on the last softmax write -- saving the
~1.4us of HWDGE+DGE fixed cost that a tail dma_start would serialize.

Software pipeline: each mean-field stage is split into y-regions that flow
through PE (conv matmuls) -> ACT (exp) -> DVE (sum/recip/mul) as a wavefront,
so region A's softmax overlaps region B's matmuls.  Because the conv needs a
5-row halo, a region's upper boundary must drift down by 5 rows per stage;
boundaries are drifted accordingly (bounds(k) below).  The stage-0 softmax
uses its own (finer) split so the first conv region is unblocked earlier.

Sharding: the whole problem is ~15us of work dominated by per-instruction
overheads, so cross-core communication would cost more than it saves; every
core runs the identical full-image program (SPMD replication) and the host
takes core 0's output.
"""

import numpy as np

H = 96
W = 96
C = 3
RAD = 5            # connectivity radius (dist <= 5)
NUM_ITERS = 5
NDY = 2 * RAD + 1  # 11
YPAD = W + 2 * RAD  # padded y extent = 106
N_CORES = 8

OUT_ROW = 384           # qout DRAM row stride in fp16 elems (768B, mult of 256B)

# Region upper boundaries at stage 0 (the q0 softmax); each drifts -5 per
# stage over 6 stages (q0 + 5 iterations).
BOUNDS0 = (34, 63, 92)
DRIFT = 5
# Stage-0 softmax sub-split (independent of conv regions; Tile wires the
# sub-AP deps).  None = use _bounds(0).
SOFT0_BOUNDS = None
PSUM_BUFS = 1

_cache = {}


def _kernel_matrix():
    """K[dy+5, dx+5] = exp(-sqrt(dy^2+dx^2)) if 0 < dy^2+dx^2 <= 25 else 0."""
    k = np.zeros((NDY, NDY), np.float64)
    for dy in range(-RAD, RAD + 1):
        for dx in range(-RAD, RAD + 1):
            d2 = dy * dy + dx * dx
            if 0 < d2 <= RAD * RAD:
                k[dy + RAD, dx + RAD] = np.exp(-np.sqrt(float(d2)))
    return k


def _band_matrices():
    """band[x, j, x'] = K[j, x - x' + 5]: out[x'] = sum_x band[x,j,x'] q[x]."""
    k = _kernel_matrix()
    idx = np.arange(H)
    d = idx[:, None] - idx[None, :]          # x - x'
    band = np.zeros((H, NDY, H), np.float32)
    for j in range(NDY):
        vals = np.where(np.abs(d) <= RAD, k[j, np.clip(d + RAD, 0, NDY - 1)], 0.0)
        band[:, j, :] = vals.astype(np.float32)
    return band


def _bounds(k):
    """Region edges [0, b1-5k, ..., 96] for stage k (k=0 is the q0 softmax)."""
    bs = [0] + [b - DRIFT * k for b in BOUNDS0] + [W]
    assert all(bs[i] < bs[i + 1] for i in range(len(bs) - 1)), bs
    return bs


def _build_nc(comp):
    """Build and compile the Bass module. comp values are baked as immediates."""
    import concourse.bacc as bacc
    import concourse.tile as tile
    import concourse.bass as bass
    from concourse.instruction_name_ordered_set import InstructionNameOrderedSet
    from concourse import mybir

    f32 = mybir.dt.float32
    fp16 = mybir.dt.float16
    bf16 = mybir.dt.bfloat16
    i16 = mybir.dt.int16
    nc = bacc.Bacc("TRN2", target_bir_lowering=False, debug=False)

    # uid: row x = [unary c0..c2 | ident] (plain HWDGE DMA).  band: row x =
    # [band j0..j10 | zero pad] -- 12 rows = 2304B, a multiple of the SWDGE
    # gather's 256B elem restriction.
    uid_d = nc.dram_tensor("uid", [H, C + 1, H], fp16, kind="ExternalInput")
    band_d = nc.dram_tensor("band", [H, (NDY + 1) * H], fp16,
                            kind="ExternalInput")
    # Output: row x = 288 payload elems (c,y packed) + pad to a 768B stride.
    qout_d = nc.dram_tensor("qout", [H, OUT_ROW], fp16, kind="ExternalOutput")

    Exp = mybir.ActivationFunctionType.Exp
    Add = mybir.AluOpType.add
    Mult = mybir.AluOpType.mult

    zero_bias = [None]
    comp_is_eye = np.allclose(comp, np.eye(C))

    def bcast_c(ap):
        """[p, n]-ish AP -> [p, C(bcast), n] via a 0-stride middle dim."""
        return bass.AP(
            tensor=ap.tensor, offset=ap.offset,
            ap=[list(ap.ap[0]), [0, C], list(ap.ap[1])],
        )

    def flat_rows(ap, sz):
        """Tile base AP -> the [[p,128],[sz,1],[1,sz]] shape SWDGE expects."""
        return bass.AP(
            tensor=ap.tensor, offset=ap.offset,
            ap=[list(ap.ap[0]), [sz, 1], [1, sz]],
        )

    def softmax_region(work, logits_ap, q_write_ap, n, x, dt_):
        """q = exp(logits)/sum_c exp(logits) over an n-column y-region.

        x tags the region so tile-pool rotation keeps regions distinct.
        """
        e = work.tile([H, C, n], dt_, tag=f"e{x}")
        nc.scalar.activation(
            out=e[:, :, :], in_=logits_ap, func=Exp, bias=zero_bias[0][:, :],
        )
        s01 = work.tile([H, n], dt_, tag=f"s01{x}")
        nc.vector.tensor_tensor(out=s01[:, :], in0=e[:, 0, :], in1=e[:, 1, :], op=Add)
        s = work.tile([H, n], dt_, tag=f"s{x}")
        nc.vector.tensor_tensor(out=s[:, :], in0=s01[:, :], in1=e[:, 2, :], op=Add)
        r = work.tile([H, n], dt_, tag=f"r{x}")
        nc.vector.reciprocal(out=r[:, :], in_=s[:, :])
        nc.vector.tensor_tensor(
            out=q_write_ap, in0=e[:, :, :], in1=bcast_c(r[:, :]), op=Mult,
        )

    def softmax_mix_region(work, logits_ap, q_writes, n, x):
        """General-comp path (not exercised by the harness: comp == eye)."""
        e = work.tile([H, C, n], f32, tag=f"e{x}")
        nc.scalar.activation(
            out=e[:, :, :], in_=logits_ap, func=Exp, bias=zero_bias[0][:, :],
        )
        s01 = work.tile([H, n], f32, tag=f"s01{x}")
        nc.vector.tensor_tensor(out=s01[:, :], in0=e[:, 0, :], in1=e[:, 1, :], op=Add)
        s = work.tile([H, n], f32, tag=f"s{x}")
        nc.vector.tensor_tensor(out=s[:, :], in0=s01[:, :], in1=e[:, 2, :], op=Add)
        r = work.tile([H, n], f32, tag=f"r{x}")
        nc.vector.reciprocal(out=r[:, :], in_=s[:, :])
        for out_ap, coeffs in q_writes:
            nz = [(d, float(cv)) for d, cv in enumerate(coeffs) if cv != 0.0]
            if not nz:
                nc.vector.memset(out_ap, 0.0)
                continue
            acc = work.tile([H, n], f32, tag=f"acc{x}")
            d0, c0 = nz[0]
            nc.vector.tensor_scalar_mul(out=acc[:, :], in0=e[:, d0, :], scalar1=c0)
            for d1, c1 in nz[1:]:
                nc.vector.scalar_tensor_tensor(
                    out=acc[:, :], in0=e[:, d1, :], scalar=c1, in1=acc[:, :],
                    op0=Mult, op1=Add,
                )
            nc.vector.tensor_tensor(out=out_ap, in0=acc[:, :], in1=r[:, :], op=Mult)

    with nc.allow_low_precision(reason="fp16 softmax within 2e-2 rel tolerance"):
        with tile.TileContext(nc) as tc:
            with (
                tc.tile_pool(name="const", bufs=1) as const,
                tc.tile_pool(name="work", bufs=2) as work,
                tc.tile_pool(name="psum", bufs=PSUM_BUFS, space="PSUM") as psum,
            ):
                # --- gather indices: idx[k%16, k//16] = k for k<96; every
                # entry must be in [-1, 95] (ucode range assert), 0 is fine
                # for unused rows.  iota exists only on Pool; the bulk memset
                # of the unused partitions runs on the (idle) DVE so Pool can
                # go straight from iota to the SWDGE preps.
                idx = const.tile([128, 6], i16, tag="idx")
                nc.vector.memset(idx[:, :], 0)
                nc.gpsimd.iota(idx[0:16, :], pattern=[[16, 6]], base=0,
                               channel_multiplier=1)

                # unary+ident via plain HWDGE DMA on SP: with an otherwise
                # idle HWDGE this is the fastest first-data path (~3.1us),
                # and it gates the whole stage-0 softmax.
                uid = const.tile([H, C + 1, H], fp16, tag="uid_r")
                nc.sync.dma_start(out=uid[:, :, :], in_=uid_d[:, :, :])

                # band via SWDGE prepared gather + trigger: descriptor gen
                # runs here in the preamble; the trigger fires the transfer
                # with none of HWDGE's 625+650ns fixed cost in the chain.
                bt = const.tile([128, NDY + 1, H], fp16, tag="bt")
                band_sem = nc.alloc_semaphore("band_dma")
                out_sem = nc.alloc_semaphore("out_dma")
                prep1 = nc.gpsimd.dma_gather(
                    flat_rows(bt[:, :, :], (NDY + 1) * H), band_d[:, :],
                    idx[:, :], H, H, (NDY + 1) * H,
                    prepare_only=True, sem=band_sem,
                )
                trig1 = nc.gpsimd.trigger_dma(count=None)

                # qa/qb: only the +-5 y-padding needs zeroing; small memsets
                # (after the prep, before the first softmax writes) keep Pool
                # free early and avoid a WAW hazard with the interior writes.
                qa = const.tile([H, C, YPAD], fp16, tag="qa")
                qb = const.tile([H, C, YPAD], fp16, tag="qb")
                pad_memsets = []
                for qt in (qa, qb):
                    for sl in (qt[:, :, 0:RAD], qt[:, :, RAD + W:YPAD]):
                        pad_memsets.append(nc.gpsimd.memset(sl.bitcast(bf16), 0.0))

                u_sl = lambda lo, hi: uid[:, 0:C, lo:hi]
                ident_ap = uid[:, C, :]
                band_j = lambda j: bt[0:H, j, :]

                # Final q lives here (const pool: stable address for the
                # scatter descriptors prepped below).
                out_t = const.tile([128, C, W], fp16, tag="outt")
                prep2 = nc.gpsimd.dma_scatter_add(
                    qout_d[:, 0:C * W], flat_rows(out_t[:, :, :], C * W),
                    idx[:, :], H, H, C * W, elem_step=OUT_ROW,
                    prepare_only=True, sem=out_sem,
                )
                prep2.ins.add_nosync_dependencies_from(
                    InstructionNameOrderedSet([trig1.ins.name]))

                # PE warm-up: the cost model ramps the PE clock over ~3us from
                # the first PE activity; tiny matmuls early keep the ramp going
                # so the real conv matmuls run at full clock.
                warm_in = const.tile([128, 16], bf16, tag="warm")
                nc.vector.memset(warm_in[:, :], 0.0)
                warm_ps = psum.tile([16, 16], f32, tag="warmps")
                nc.tensor.matmul(
                    warm_ps[:, :], warm_in[:, :16], warm_in[:, :16],
                    start=True, stop=True,
                )

                # Explicit zero bias for all activations, memset early on DVE.
                zb = const.tile([H, 1], f32, tag="zb")
                nc.vector.memset(zb[:, :], 0.0)
                zero_bias[0] = zb

                # Trigger the exp table load (+its drain) immediately so the
                # first real softmax doesn't pay the ~2.6us load.
                warm_act = const.tile([1, 1], f32, tag="warmact")
                nc.vector.memset(warm_act[:, :], 0.0)
                nc.scalar.activation(
                    out=warm_act[:, :], in_=warm_act[:, :], func=Exp,
                    bias=zb[:1, :],
                )

                # Zero the padded qout DRAM rows via a plain HWDGE DMA on the
                # ACT engine (its SEQ is busy with the exp-table load until
                # ~2us, which conveniently sequences this transfer after the
                # band gather on the shared DMA engines); the scatter-ADD at
                # the end then writes q onto the zeros.
                zeros = const.tile([H, OUT_ROW], fp16, tag="zeros")
                nc.vector.memset(zeros[:, :].bitcast(f32), 0.0)
                zdma = nc.scalar.dma_start(out=qout_d[:, :], in_=zeros[:, :])
                zero_done = nc.alloc_semaphore("zero_done")
                zdma.then_inc(zero_done, 16)

                mix_writes = lambda qt, lo, hi: [
                    (qt[:, c, RAD + lo:RAD + hi], [comp[c, d] for d in range(C)])
                    for c in range(C)
                ]

                # stage 0: q0 = softmax(unary), region by region
                if comp_is_eye:
                    bs0 = list(SOFT0_BOUNDS) if SOFT0_BOUNDS else _bounds(0)
                else:
                    bs0 = [0, W]
                for x in range(len(bs0) - 1):
                    lo, hi = bs0[x], bs0[x + 1]
                    n = hi - lo
                    if comp_is_eye:
                        softmax_region(
                            work, u_sl(lo, hi),
                            qa[:, :, RAD + lo:RAD + hi], n, f"z{x}", fp16,
                        )
                    else:
                        softmax_mix_region(
                            work, u_sl(lo, hi), mix_writes(qa, lo, hi), n, x,
                        )

                cur, nxt = qa, qb
                for t in range(NUM_ITERS):
                    last = t == NUM_ITERS - 1
                    bs = _bounds(t + 1) if comp_is_eye else [0, W]
                    ms = []
                    # PE: per-region conv matmul blocks (in region order)
                    for x in range(len(bs) - 1):
                        lo, hi = bs[x], bs[x + 1]
                        n = hi - lo
                        m = psum.tile([H, C, n], f32, tag=f"m{x}")
                        # unary-add first: it only depends on u, so the PE can
                        # run it during the preceding softmax instead of idling.
                        nc.tensor.matmul(
                            m[:, :, :], ident_ap, u_sl(lo, hi),
                            start=True, stop=False,
                        )
                        for j in range(NDY):
                            nc.tensor.matmul(
                                m[:, :, :],
                                band_j(j),
                                cur[:, :, lo + j:hi + j],
                                start=False,
                                stop=(j == NDY - 1),
                            )
                        ms.append(m)
                    # ACT + DVE: per-region softmax, same region order
                    for x in range(len(bs) - 1):
                        lo, hi = bs[x], bs[x + 1]
                        n = hi - lo
                        m = ms[x]
                        if last:
                            if comp_is_eye:
                                softmax_region(
                                    work, m[:, :, :], out_t[0:H, :, lo:hi],
                                    n, x, fp16,
                                )
                            else:
                                softmax_mix_region(
                                    work, m[:, :, :],
                                    [(out_t[0:H, c, lo:hi],
                                      [1.0 if d == c else 0.0 for d in range(C)])
                                     for c in range(C)], n, x,
                                )
                        else:
                            if comp_is_eye:
                                softmax_region(
                                    work, m[:, :, :],
                                    nxt[:, :, RAD + lo:RAD + hi], n, x, fp16,
                                )
                            else:
                                softmax_mix_region(
                                    work, m[:, :, :], mix_writes(nxt, lo, hi),
                                    n, x,
                                )
                    cur, nxt = nxt, cur

                # Fire the output scatter: waits on the last out_t writes (and
                # the DRAM zero-fill), then the DMA engines stream it out.
                nc.gpsimd.wait_ge(zero_done, 16)
                nc.gpsimd.trigger_dma(count=None)

    # Tile assigns each SWDGE prep a DMASW lane and makes data consumers (and
    # the end-of-module barrier) wait on that lane's semaphore.  On hardware
    # the lane sem is released by the IncSwdgeSem/ring-drain protocol, which
    # the no_exec cost model cannot see -- TimelineSim only fires the prep's
    # on_update[0] (the user `sem=`, which the DMA descriptors also bump for
    # real on hardware).  Rewrite the DMASW waits to wait on the user sem
    # instead: both hardware and the cost model fire that one at data
    # completion, and the module's behavior is unchanged otherwise.
    fn = nc.m.functions[0]
    lane_user = {}
    lane = 0
    for b in fn.blocks:
        for inst in b.instructions:
            if getattr(inst, "gen_mode", 0) == 1:
                u0 = inst.sync_info.on_update[0]
                lane_user[lane] = (u0.id, u0.ant_name)
                lane += 1
    for b in fn.blocks:
        for inst in b.instructions:
            si = inst.sync_info
            if not si:
                continue
            ws = si.on_wait
            changed = False
            new_ws = []
            for w in ws:
                nm = w.ant_name or ""
                if nm.startswith("DMASW"):
                    sid, snm = lane_user[int(nm[5:].split("_")[0])]
                    new_ws.append(mybir.SyncWait(
                        sync_type=w.sync_type, id=sid, ant_name=snm,
                        wait_mode=w.wait_mode, wait_value=w.wait_value,
                        wait_reg=None,
                    ))
                    changed = True
                else:
                    new_ws.append(w)
            if changed:
                si.on_wait = new_ws

    nc.compile()
    return nc


def get_nc(comp):
    key = (comp.tobytes(), BOUNDS0, DRIFT, tuple(SOFT0_BOUNDS or ()), PSUM_BUFS)
    if key not in _cache:
        _cache[key] = _build_nc(comp)
    return _cache[key]


def make_inputs(unary):
    """Host-side layout prep: unary [1,C,H,W] (c,y,x) -> [x, c, y], plus the
    (input-independent) identity and band-matrix gather rows."""
    uid = np.empty((H, C + 1, H), np.float16)
    uid[:, 0:C, :] = np.transpose(unary[0], (2, 0, 1)).astype(np.float16)
    uid[:, C, :] = np.eye(H, dtype=np.float16)
    band = np.zeros((H, NDY + 1, H), np.float16)
    band[:, 0:NDY, :] = _band_matrices().astype(np.float16)
    return {"uid": uid, "band": band.reshape(H, (NDY + 1) * H)}


def kernel(**inputs):
    from concourse.bass_utils import run_bass_kernel_spmd

    unary = np.asarray(inputs["unary"], dtype=np.float32)
    comp = np.asarray(inputs["compatibility"], dtype=np.float32)
    assert unary.shape == (1, C, H, W), unary.shape

    nc = get_nc(comp)
    in_map = make_inputs(unary)
    res = run_bass_kernel_spmd(
        nc, [dict(in_map) for _ in range(N_CORES)], core_ids=list(range(N_CORES)),
    )
    q = np.asarray(res.results[0]["qout"], dtype=np.float32)   # [x, 384]
    q = q[:, :C * W].reshape(H, C, W)                          # [x, c, y]
    out = np.transpose(q, (1, 2, 0))[None]                     # [1, c, y, x]
    return np.ascontiguousarray(out.astype(np.float32))
